# revision 1
# baseline (speedup 1.0000x reference)
"""AutoCorrelation (FFT cross-correlation + full-sort delay aggregation) on 8 NeuronCores.

Math (per batch b, channels c = (h,e), C = 512, L = 512):
  mv[t]   = (1/C) sum_c irfft( Q_c * conj(K_c) )[t]        (channel-mean correlation)
  rank0   = descending ranks of mv[batch 0]
  g[b, j] = softmax(mv[b])_sorted[ rank0[j] ]              (rank-matched scatter)
  out[b,t,c] = sum_u g[b,u] * v[b,(t+u) % L, c]            (circular correlation)

Device mapping per core (4 local batches + a redundant batch-0 slot):
  - rfft of q,k as matmuls against cos/sin DFT matrices (fp32r, full PE rate)
  - S_r/S_i spectra products + channel reduction fused in DVE scalar_tensor_tensor accum
  - irfft as matmul with inverse-DFT constant
  - ranks via ACT Sign(+bias) with accumulate; rank matching via DVE is_equal compare
  - softmax via ACT Exp (+accumulate for Z); no max-subtraction needed (|mv| small)
  - circulant of g materialized via negative-stride DMA from a doubled DRAM copy
  - aggregation as circulant matmul against v in natural layout
"""

import sys, os
for _p in ('/opt/trn_rl_repo',):
    if _p not in sys.path:
        sys.path.insert(0, _p)

import numpy as np
from contextlib import ExitStack

import concourse.bass as bass
import concourse.bacc as bacc
import concourse.tile as tile
import concourse.mybir as mybir
from concourse.bass_utils import run_bass_kernel_spmd

F32 = mybir.dt.float32
F32R = mybir.dt.float32r
AL = mybir.AluOpType
AF = mybir.ActivationFunctionType

B, L, H, E = 32, 512, 8, 64
C = H * E          # 512 channels per batch
NCORES = 8
NB = B // NCORES   # 4 local batches per core
NSLOT = NB + 1     # + redundant batch-0 slot


def _consts():
    l = np.arange(L)[:, None].astype(np.float64)
    f = np.arange(257)[None, :].astype(np.float64)
    Wc = np.cos(2 * np.pi * l * f / L).astype(np.float32)            # [512, 257]
    Ws = np.sin(2 * np.pi * l * f[:, :256] / L).astype(np.float32)   # [512, 256]
    m = np.arange(L)[None, :].astype(np.float64)
    fc = np.arange(257)[:, None].astype(np.float64)
    wgt = np.where((fc == 0) | (fc == 256), 1.0, 2.0)
    Ar = (wgt * np.cos(2 * np.pi * fc * m / L) / (L * C)).astype(np.float32)   # [257, 512]
    fs = np.arange(256)[:, None].astype(np.float64)
    wgt_i = np.where(fs == 0, 0.0, 2.0)
    Ai = (-wgt_i * np.sin(2 * np.pi * fs * m / L) / (L * C)).astype(np.float32)  # [256, 512]
    # Pack the Nyquist (f=256) cos transform into Ws's all-zero f=0 column:
    # the sin-half partition 0 then carries Q_N/K_N, whose S_r product pairs
    # with Ar's Nyquist row via ArB0; its S_i garbage hits Ai's zero f=0 row.
    Ws = Ws.copy()
    Ws[:, 0] = np.cos(np.pi * np.arange(L)).astype(np.float32)
    ArB0 = Ar[0:128].copy()
    ArB0[0] = Ar[256]
    return Wc, Ws, Ar, Ai, ArB0


_NC_CACHE = None


def _build():
    global _NC_CACHE
    if _NC_CACHE is not None:
        return _NC_CACHE
    Wc_np, Ws_np, Ar_np, Ai_np, ArB0_np = _consts()

    nc = bacc.Bacc("TRN2", target_bir_lowering=False, debug=False, num_devices=NCORES)
    tc = tile.TileContext(nc)

    q_all = nc.dram_tensor("q_all", [NSLOT, L, C], F32R, kind="ExternalInput")
    k_all = nc.dram_tensor("k_all", [NSLOT, L, C], F32R, kind="ExternalInput")
    v_all = nc.dram_tensor("v_all", [NB, L, C], F32R, kind="ExternalInput")
    out_all = nc.dram_tensor("out_all", [NB, L, C], F32, kind="ExternalOutput")

    Wc_d = nc.inline_tensor(Wc_np.view(np.float32), "Wc_d")
    Ws_d = nc.inline_tensor(Ws_np, "Ws_d")
    Ar_d = nc.inline_tensor(Ar_np, "Ar_d")
    Ai_d = nc.inline_tensor(Ai_np, "Ai_d")
    two_d = nc.inline_tensor(np.full((128, 1), 2.0, np.float32), "two_d")
    one_d = nc.inline_tensor(np.ones((1, 1), np.float32), "one_d")
    ArB0_d = nc.inline_tensor(ArB0_np, "ArB0_d")
    antiI_np = np.zeros((128, 128), np.float32)
    antiI_np[np.arange(128), 127 - np.arange(128)] = 1.0
    antiI_d = nc.inline_tensor(antiI_np, "antiI_d")

    with tc, ExitStack() as ctx:
        cpool = ctx.enter_context(tc.tile_pool(name="consts", bufs=1))
        iopool = ctx.enter_context(tc.tile_pool(name="io", bufs=1))
        wpool = ctx.enter_context(tc.tile_pool(name="work", bufs=1))
        spool = ctx.enter_context(tc.tile_pool(name="scol", bufs=1))
        pspec = ctx.enter_context(tc.tile_pool(name="pspec", bufs=1, space="PSUM"))
        psmall = ctx.enter_context(tc.tile_pool(name="psmall", bufs=1, space="PSUM"))
        dpool = ctx.enter_context(tc.tile_pool(name="dscratch", bufs=1, space="DRAM"))

        # ---- constants to SBUF ----
        Wc_t = []
        Ws_t = []
        for lc in range(4):
            t = cpool.tile([128, 257], F32R, name=f"Wc_t{lc}")
            nc.sync.dma_start(t[:], bass.AP(tensor=Wc_d, offset=lc * 128 * 257, ap=[[257, 128], [1, 257]]).bitcast(F32R))
            Wc_t.append(t)
            t2 = cpool.tile([128, 256], F32R, name=f"Ws_t{lc}")
            nc.sync.dma_start(t2[:], bass.AP(tensor=Ws_d, offset=lc * 128 * 256, ap=[[256, 128], [1, 256]]).bitcast(F32R))
            Ws_t.append(t2)
        Ar_t = [cpool.tile([128, 512], F32R, name=f"Ar_t{g}") for g in range(2)]
        Ai_t = [cpool.tile([128, 512], F32R, name=f"Ai_t{g}") for g in range(2)]
        ArB0_t = cpool.tile([128, 512], F32R, name="ArB0_t")
        two_t = cpool.tile([128, 1], F32R, name="two_t")
        one_t = cpool.tile([1, 1], F32, name="one_t")
        antiI_t = cpool.tile([128, 128], F32, name="antiI_t")
        n2bBR = cpool.tile([128, 512], F32, name="n2bBR")  # reversed (2*rank0-511) broadcast
        late_consts = [False]

        def P(bi):
            bi.ins.bass_priority = -50
            return bi

        def load_late_consts():
            # Emitted after the first slot's FFT matmuls so these 1.3MB don't
            # delay the urgent Wc/Ws/q/k loads at kernel start.
            if late_consts[0]:
                return
            late_consts[0] = True
            for g in range(2):
                nc.sync.dma_start(Ar_t[g][:], bass.AP(tensor=Ar_d, offset=g * 128 * 512, ap=[[512, 128], [1, 512]]).bitcast(F32R))
                nc.sync.dma_start(Ai_t[g][:], bass.AP(tensor=Ai_d, offset=g * 128 * 512, ap=[[512, 128], [1, 512]]).bitcast(F32R))
            nc.sync.dma_start(ArB0_t[:], ArB0_d.ap().bitcast(F32R))
            nc.sync.dma_start(two_t[:], two_d.ap().bitcast(F32R))
            nc.sync.dma_start(one_t[:], one_d.ap())
            nc.sync.dma_start(antiI_t[:], antiI_d.ap())

        # ---------- per-slot stage A: mv row ----------
        def stage_mv(s):
            q_t, k_t = [], []
            for lc in range(4):
                t = iopool.tile([128, 512], F32R, name=f"q_s{s}_l{lc}", tag=f"qt{lc}", bufs=2)
                nc.sync.dma_start(t[:], q_all[s, 128 * lc:128 * (lc + 1), :])
                q_t.append(t)
                t = iopool.tile([128, 512], F32R, name=f"k_s{s}_l{lc}", tag=f"kt{lc}", bufs=2)
                nc.sync.dma_start(t[:], k_all[s, 128 * lc:128 * (lc + 1), :])
                k_t.append(t)

            scols = []   # (tile, rhs_const) pairs for the irfft matmul
            for g in range(2):
                sq = pspec.tile([128, 1024], F32, name=f"specq_s{s}g{g}", tag="spec", bufs=3)
                sk = pspec.tile([128, 1024], F32, name=f"speck_s{s}g{g}", tag="spec", bufs=3)
                for (spec, src) in ((sq, q_t), (sk, k_t)):
                    for lc in range(4):
                        nc.tensor.matmul(spec[:, 0:512], Wc_t[lc][:, 128 * g:128 * (g + 1)], src[lc][:],
                                         start=(lc == 0), stop=(lc == 3))
                    for lc in range(4):
                        nc.tensor.matmul(spec[:, 512:1024], Ws_t[lc][:, 128 * g:128 * (g + 1)], src[lc][:],
                                         start=(lc == 0), stop=(lc == 3))
                sk_sb = wpool.tile([128, 1024], F32, name=f"sksb_s{s}g{g}", tag="sksb", bufs=2)
                nc.scalar.copy(sk_sb[:, 0:512], sk[:, 0:512])
                nc.scalar.copy(sk_sb[:, 512:1024], sk[:, 512:1024])
                scr = wpool.tile([128, 1024], F32R, name=f"sttscr_s{s}g{g}", tag="sttscr", bufs=2)
                si1 = spool.tile([128, 1], F32R, name=f"si1_s{s}g{g}", tag=f"si1{g}", bufs=2)
                si2 = spool.tile([128, 1], F32R, name=f"si2_s{s}g{g}", tag=f"si2{g}", bufs=2)
                si = spool.tile([128, 1], F32R, name=f"si_s{s}g{g}", tag=f"si{g}", bufs=2)
                if g == 0:
                    # cos-half and sin-half reduced separately: sin-half partition 0
                    # carries the Nyquist product and pairs with ArB0's Nyquist row.
                    srA = spool.tile([128, 1], F32R, name=f"srA_s{s}", tag="srA", bufs=2)
                    srB = spool.tile([128, 1], F32R, name=f"srB_s{s}", tag="srB", bufs=2)
                    nc.vector.scalar_tensor_tensor(scr[:, 0:512], sq[:, 0:512], 1.0, sk_sb[:, 0:512], AL.mult, AL.mult, accum_out=srA[:])
                    nc.vector.scalar_tensor_tensor(scr[:, 512:1024], sq[:, 512:1024], 1.0, sk_sb[:, 512:1024], AL.mult, AL.mult, accum_out=srB[:])
                    scols.append((srA, Ar_t[0]))
                    scols.append((srB, ArB0_t))
                else:
                    sr = spool.tile([128, 1], F32R, name=f"sr_s{s}g{g}", tag=f"sr{g}", bufs=2)
                    nc.vector.scalar_tensor_tensor(scr[:], sq[:, 0:1024], 1.0, sk_sb[:, 0:1024], AL.mult, AL.mult, accum_out=sr[:])
                    scols.append((sr, Ar_t[g]))
                # S_i = sum_c Qr*Ks - Qs*Kr  (f=0 garbage killed by Ai's zero row)
                nc.vector.scalar_tensor_tensor(scr[:, 0:512], sq[:, 0:512], 1.0, sk_sb[:, 512:1024], AL.mult, AL.mult, accum_out=si1[:])
                nc.vector.scalar_tensor_tensor(scr[:, 0:512], sq[:, 512:1024], 1.0, sk_sb[:, 0:512], AL.mult, AL.mult, accum_out=si2[:])
                nc.vector.tensor_sub(si[:], si1[:].bitcast(F32), si2[:].bitcast(F32))
                scols.append((si, Ai_t[g]))

            load_late_consts()
            mv_ps = psmall.tile([1, 512], F32, name=f"mvps_s{s}", tag="psm", bufs=1)
            n = len(scols)
            for i, (scol, rhs) in enumerate(scols):
                nc.tensor.matmul(mv_ps[:], scol[:], rhs[:], start=(i == 0), stop=(i == n - 1))
            mv_sb = wpool.tile([1, 512], F32, name=f"mvsb_s{s}", tag="mvsb", bufs=2)
            P(nc.scalar.copy(mv_sb[:], mv_ps[:]))
            mvB = wpool.tile([128, 512], F32, name=f"mvB_s{s}", tag="mvB", bufs=2)
            P(nc.gpsimd.partition_broadcast(mvB[:], mv_sb[:]))
            mvT_ps = psmall.tile([128, 4], F32, name=f"mvT_s{s}", tag="psm", bufs=1)
            for j in range(4):
                nc.tensor.transpose(mvT_ps[:, j:j + 1], mv_sb[0:1, 128 * j:128 * (j + 1)], one_t[:])
            return mv_sb, mvB, mvT_ps

        # ---------- slot 4: batch-0 ranks ----------
        mv_sb0, mvB0, mvT_ps0 = stage_mv(4)
        posmvT = wpool.tile([128, 4], F32, name="posmvT", bufs=1)
        nc.scalar.copy(posmvT[:], mvT_ps0[:])
        # reversed mv0 row via anti-identity matmuls on the column chunks
        mv0r_ps = psmall.tile([1, 512], F32, name="mv0r_ps", tag="psm", bufs=1)
        for j in range(4):
            nc.tensor.matmul(mv0r_ps[0:1, 128 * (3 - j):128 * (4 - j)], posmvT[:, j:j + 1], antiI_t[:],
                             start=True, stop=True)
        mv0r_sb = wpool.tile([1, 512], F32, name="mv0r_sb", bufs=1)
        nc.scalar.copy(mv0r_sb[:], mv0r_ps[:])
        mvB0R = wpool.tile([128, 512], F32, name="mvB0R", bufs=1)
        nc.gpsimd.partition_broadcast(mvB0R[:], mv0r_sb[:])
        r2_ps = psmall.tile([1, 512], F32, name="r2_ps", tag="psm", bufs=1)
        for j in range(4):
            c2 = wpool.tile([128, 512], F32R, name=f"c2_{j}", tag="c2", bufs=2)
            nc.vector.tensor_scalar(c2[:], mvB0R[:], posmvT[:, j:j + 1], None, AL.is_lt)
            nc.tensor.matmul(r2_ps[:], two_t[:], c2[:], start=(j == 0), stop=(j == 3))
        n2b_row = wpool.tile([1, 512], F32, name="n2b_row", bufs=1)
        nc.scalar.activation(n2b_row[:], r2_ps[:], AF.Copy, bias=-511.0, scale=1.0)
        nc.gpsimd.partition_broadcast(n2bBR[:], n2b_row[:])

        # ---------- local slots ----------
        for s in range(NB):
            mv_sb, mvB, mvT_ps = stage_mv(s)
            negmvT = wpool.tile([128, 4], F32, name=f"negmvT_{s}", tag="negmvT", bufs=2)
            P(nc.scalar.activation(negmvT[:], mvT_ps[:], AF.Copy, bias=0.0, scale=-1.0))
            # ranksign[i] = sum_m Sign(mv[m] - mv[i])  = 511 - 2*rankL[i]
            # W[i,m] = [desc_rank_b[i] == desc_rank_0[m]]  <=>  ranksign[i] == -(2*rank0[m]-511)
            # n2bB holds 2*rank0[m]-511, so compare ranksign vs -n2bB -> use negated sign accum
            rs = wpool.tile([128, 4], F32, name=f"rs_{s}", tag="rs", bufs=2)
            sgnscr = wpool.tile([128, 512], F32, name=f"sgnscr_{s}", tag="sgnscr", bufs=2)
            for j in range(4):
                P(nc.scalar.activation(sgnscr[:], mvB[:], AF.Sign, bias=negmvT[:, j:j + 1], accum_out=rs[:, j:j + 1]))
            # negate ranksign so it equals 2*rank_desc[i]-511:  -(511-2rankL)= 2rankL-511;
            # careful: rank_desc = 511 - rankL  =>  2*rank_desc-511 = 511-2rankL = +ranksign.
            # So compare ranksign[i] == n2bB[m] directly.
            expz = wpool.tile([1, 512], F32, name=f"expz_{s}", tag="expz", bufs=2)
            z_sb = wpool.tile([1, 1], F32, name=f"z_{s}", tag="z", bufs=2)
            nc.scalar.activation(expz[:], mv_sb[:], AF.Exp, accum_out=z_sb[:])
            rz = wpool.tile([1, 1], F32, name=f"rz_{s}", tag="rz", bufs=2)
            nc.vector.reciprocal(rz[:], z_sb[:])
            smc = wpool.tile([128, 4], F32, name=f"smc_{s}", tag="smc", bufs=2)
            for j in range(4):
                nc.scalar.activation(smc[:, j:j + 1], mvT_ps[:, j:j + 1], AF.Exp)
            smcr = wpool.tile([128, 4], F32R, name=f"smcr_{s}", tag="smcr", bufs=2)
            nc.vector.tensor_copy(smcr[:], smc[:])

            g_ps = psmall.tile([1, 512], F32, name=f"gps_{s}", tag="psm", bufs=1)
            for j in range(4):
                wt = wpool.tile([128, 512], F32R, name=f"wt_{s}_{j}", tag=f"wt{j}", bufs=2)
                P(nc.vector.tensor_scalar(wt[:], n2bBR[:], rs[:, j:j + 1], None, AL.is_equal))
                P(nc.tensor.matmul(g_ps[:], smcr[:, j:j + 1], wt[:], start=(j == 0), stop=(j == 3)))
            gn = wpool.tile([1, 512], F32, name=f"gn_{s}", tag="gn", bufs=2)
            P(nc.scalar.activation(gn[:], g_ps[:], AF.Copy, bias=0.0, scale=rz[:]))
            gRB = wpool.tile([128, 512], F32, name=f"gRB_{s}", tag="gRB", bufs=2)
            P(nc.gpsimd.partition_broadcast(gRB[:], gn[:]))
            gmatR = dpool.tile([128, 1024], F32R, name=f"gmatR_{s}", tag="gmatR", bufs=2)
            nc.sync.dma_start(gmatR[:, 0:512], gRB[:].bitcast(F32R))
            nc.sync.dma_start(gmatR[:, 512:1024], gRB[:].bitcast(F32R))

            # ---------- stage C ----------
            v_t = []
            for lc in range(4):
                t = iopool.tile([128, 512], F32R, name=f"v_s{s}_l{lc}", tag=f"vt{lc}", bufs=2)
                nc.sync.dma_start(t[:], v_all[s, 128 * lc:128 * (lc + 1), :])
                v_t.append(t)
            cg_t = []
            gd_handle = gmatR[:].tensor
            for ss in range(4):
                t = iopool.tile([128, 512], F32R, name=f"cg_s{s}_c{ss}", tag=f"cg{ss}", bufs=2)
                srcap = bass.AP(tensor=gd_handle, offset=511 - 128 * ss, ap=[[1023, 128], [1, 512]])
                nc.sync.dma_start(t[:], srcap)
                cg_t.append(t)
            for tt in range(4):
                o_ps = psmall.tile([128, 512], F32, name=f"ops_{s}_{tt}", tag="outp", bufs=1)
                for ss in range(4):
                    nc.tensor.matmul(o_ps[:], cg_t[ss][:, 128 * tt:128 * (tt + 1)], v_t[ss][:],
                                     start=(ss == 0), stop=(ss == 3))
                o_sb = wpool.tile([128, 512], F32, name=f"osb_{s}_{tt}", tag=f"osb{tt}", bufs=2)
                nc.vector.tensor_copy(o_sb[:], o_ps[:])
                nc.sync.dma_start(out_all[s, 128 * tt:128 * (tt + 1), :], o_sb[:])

    nc.compile()
    _NC_CACHE = nc
    return nc


def kernel(queries, keys, values):
    q = np.ascontiguousarray(queries, dtype=np.float32).reshape(B, L, C)
    k = np.ascontiguousarray(keys, dtype=np.float32).reshape(B, L, C)
    v = np.ascontiguousarray(values, dtype=np.float32).reshape(B, L, C)
    nc = _build()
    in_maps = []
    for c in range(NCORES):
        sl = slice(NB * c, NB * (c + 1))
        in_maps.append({
            "q_all": np.concatenate([q[sl], q[0:1]], axis=0),
            "k_all": np.concatenate([k[sl], k[0:1]], axis=0),
            "v_all": v[sl],
        })
    res = run_bass_kernel_spmd(nc, in_maps, core_ids=list(range(NCORES)))
    out = np.concatenate([res.results[c]["out_all"] for c in range(NCORES)], axis=0)
    return out.reshape(B, L, H, E)


if __name__ == "__main__":
    rng = np.random.default_rng(0)
    qq = rng.standard_normal((B, L, H, E)).astype(np.float32)
    kk = rng.standard_normal((B, L, H, E)).astype(np.float32)
    vv = rng.standard_normal((B, L, H, E)).astype(np.float32)
    o = kernel(queries=qq, keys=kk, values=vv)
    print(o.shape, o.dtype, np.abs(o).max())

